# revision 12
# baseline (speedup 1.0000x reference)
"""CentroidTripletLoss Trainium2 kernel (8 NeuronCores, feature-dim sharded).

Math (matches the reference):
    centroids[c] = mean of inputs with target c           (segment mean)
    rest[c]      = (sum_c' centroids[c'] - centroids[c]) / (C-1)
    d_ap[b] = ||x_b - centroids[t_b]||,  d_an[b] = ||x_b - rest[t_b]||
    loss = mean(relu(d_ap - d_an + MARGIN))

Distribution: the feature dim D=2048 is sharded 8 ways (256 per core).
Each core computes complete per-class sums for its feature slice (no
centroid all-reduce needed), then per-sample partial squared distances;
a single 64KB AllReduce combines the partials, after which every core
finishes the (tiny) scalar loss reduction redundantly.

Two compiled variants share this builder:
  * sorted_fast: targets are exactly arange(B)//(B//C) (the identity-
    balanced sampler in the reference).  The per-chunk one-hot matrices
    are then compile-time constants (16 distinct patterns), the segment
    sum is one bf16 matmul per chunk, and the per-sample [centroid|rest]
    rows are produced by TensorE from a chunk-major SBUF table
    (diff = E @ [cent|rest] + I @ [-x|-x], accumulated in PSUM), so no
    gather DMA exists at all.
  * general: any targets in [0, C).  fp32 one-hot x 4 class-group
    matmuls, row gather via indirect DMA, VectorE subtracts.
The host picks the variant per call, so arbitrary inputs stay correct.

d_ap - d_an is evaluated as (sap - san) / (sqrt(sap) + sqrt(san)) so the
loose HW sqrt (large ULP budget) only perturbs the result by its own
relative error instead of being amplified by cancellation.
"""

from contextlib import ExitStack

import numpy as np

import concourse.bacc as bacc
import concourse.bass as bass
import concourse.tile as tile
from concourse import mybir
from concourse.bass import IndirectOffsetOnAxis
from concourse.bass_utils import run_bass_kernel_spmd

N_CORES = 8
B = 8192
D = 2048
DS = D // N_CORES  # 256 features per core
C = 512
K = B // C  # 16 samples per class when identity-balanced
NCH = B // 128  # 64 chunks of 128 samples
CG = C // 128  # 4 class groups
CW = DS + 1  # chunk width in resident fp32 X tile (features + ones col)
MARGIN = 0.3

F32 = mybir.dt.float32
BF16 = mybir.dt.bfloat16
I32 = mybir.dt.int32


def _ar(nc, drpool, src_ap, n_cores, stage, name):
    """AllReduce a [128, 64] f32 slab; returns the output DRAM tile."""
    cc_in = drpool.tile([128, NCH], F32, name=f"cc_in_{name}")
    cc_out = drpool.tile([128, NCH], F32, name=f"cc_out_{name}")
    nc.sync.dma_start(cc_in[:], src_ap)
    if stage >= 5:
        nc.gpsimd.collective_compute(
            "AllReduce",
            mybir.AluOpType.add,
            replica_groups=[list(range(n_cores))],
            ins=[cc_in.opt()],
            outs=[cc_out.opt()],
        )
    else:
        nc.sync.dma_start(cc_out[:], cc_in[:])
    return cc_out


def _loss_tail(nc, spool, ppool2, drpool, dbg_dram, out_sb, cc_out1, cc_out2,
               stage):
    """Finish the scalar loss from the two AllReduced interleaved slabs."""
    ones_f = spool.tile([128, 1], F32, tag="ones_f")
    nc.vector.memset(ones_f[:], 1.0)
    sres = spool.tile([128, 2 * NCH], F32, tag="sres")
    nc.sync.dma_start(sres[:, 0:NCH], cc_out1[:])
    nc.sync.dma_start(sres[:, NCH : 2 * NCH], cc_out2[:])
    s3 = sres[:].rearrange("p (c two) -> p c two", two=2)
    sapg = spool.tile([128, NCH], F32, tag="sapg")
    sang = spool.tile([128, NCH], F32, tag="sang")
    nc.vector.tensor_copy(sapg[:], s3[:, :, 0:1])
    nc.vector.tensor_copy(sang[:], s3[:, :, 1:2])
    nc.scalar.dma_start(dbg_dram.ap()[:, 0:NCH], sapg[:])
    nc.scalar.dma_start(dbg_dram.ap()[:, NCH : 2 * NCH], sang[:])
    dapf = spool.tile([128, NCH], F32, tag="dapf")
    danf = spool.tile([128, NCH], F32, tag="danf")
    nc.scalar.sqrt(dapf[:], sapg[:])
    nc.scalar.sqrt(danf[:], sang[:])
    num = spool.tile([128, NCH], F32, tag="num")
    den = spool.tile([128, NCH], F32, tag="den")
    nc.vector.tensor_tensor(num[:], sapg[:], sang[:], op=mybir.AluOpType.subtract)
    nc.vector.tensor_tensor(den[:], dapf[:], danf[:], op=mybir.AluOpType.add)
    rden = spool.tile([128, NCH], F32, tag="rden")
    nc.vector.reciprocal(rden[:], den[:])
    delta = spool.tile([128, NCH], F32, tag="delta")
    nc.vector.tensor_tensor(delta[:], num[:], rden[:], op=mybir.AluOpType.mult)
    terms = spool.tile([128, NCH], F32, tag="terms")
    lcol = spool.tile([128, 1], F32, tag="lcol")
    margin_t = spool.tile([128, 1], F32, tag="margin")
    nc.vector.memset(margin_t[:], MARGIN)
    nc.scalar.activation(
        terms[:],
        delta[:],
        mybir.ActivationFunctionType.Relu,
        bias=margin_t[:, 0:1],
        scale=1.0,
        accum_out=lcol[:, 0:1],
    )
    loss_ps = ppool2.tile([1, 1], F32, tag="loss")
    nc.tensor.matmul(
        loss_ps[:], lhsT=ones_f[:, 0:1], rhs=lcol[:, 0:1], start=True, stop=True
    )
    nc.scalar.mul(out_sb[:], loss_ps[:], 1.0 / B)


def build_sorted(stage=5, n_cores=N_CORES):
    """Fast path: targets == arange(B)//K (verified on host).

    Algebraic form: with q=|x|^2, cent=S/K, rest=(T-cent)/(C-1):
        sap = q - 2 x.cent[t] + |cent[t]|^2
        san = q - 2 x.rest[t] + |rest[t]|^2
    Ships X feature-major (xt[f, b]); centroids come from a segmented DVE
    reduce (16 consecutive samples per class), q from per-chunk matmuls of
    squared xt against ones, the dots from per-chunk matmuls against a
    [-2cent | -2rest] table, class selection via a masked DVE reduce, and
    the per-class norm constants via a tiny E8 matmul. All pieces are
    linear in the feature shard, so one fp32 AllReduce of [128, 128]
    (split in 2 to overlap) combines the 8 cores.
    """
    nc = bacc.Bacc(None, target_bir_lowering=False, debug=False, num_devices=n_cores)
    NG = 8  # DMA/compute groups (8 chunks each)
    GC = NCH // NG  # 8 chunks per group
    FB = 2  # feature blocks of 128
    xt_dram = nc.dram_tensor("xt", [128, FB * B], BF16, kind="ExternalInput")
    loss_dram = nc.dram_tensor("loss", [1, 1], F32, kind="ExternalOutput")
    dbg_dram = nc.dram_tensor("dbg", [128, 2 * NCH], F32, kind="ExternalOutput")

    with tile.TileContext(nc) as tc, ExitStack() as top:
        cpool = top.enter_context(tc.tile_pool(name="const", bufs=1))
        spool = top.enter_context(tc.tile_pool(name="small", bufs=1))
        qpool = top.enter_context(tc.tile_pool(name="qps", bufs=1, space="PSUM"))
        gpool = top.enter_context(tc.tile_pool(name="gps", bufs=1, space="PSUM"))
        ppool2 = top.enter_context(tc.tile_pool(name="psum2", bufs=1, space="PSUM"))
        drpool = top.enter_context(tc.tile_pool(name="dram", bufs=1, space="DRAM"))

        # ---------- constants ----------
        pcol_i = cpool.tile([128, 1], I32, tag="pcol_i")
        nc.gpsimd.iota(pcol_i[:], pattern=[[0, 1]], base=0, channel_multiplier=1)
        p16_i = cpool.tile([128, 1], I32, tag="p16_i")
        nc.vector.tensor_scalar(
            p16_i[:], pcol_i[:], 4, None, mybir.AluOpType.arith_shift_right
        )
        p16_f = cpool.tile([128, 1], F32, tag="p16_f")
        nc.vector.tensor_copy(p16_f[:], p16_i[:])
        jrow8 = cpool.tile([128, 8], F32, tag="jrow8")
        nc.gpsimd.iota(
            jrow8[:], pattern=[[1, 8]], base=0, channel_multiplier=0,
            allow_small_or_imprecise_dtypes=True,
        )
        # mask8[p, j] = (j == p//16), fp32 for the masked PSUM reduce
        mask8 = cpool.tile([128, 8], F32, tag="mask8")
        nc.vector.tensor_scalar(
            mask8[:], jrow8[:], p16_f[:, 0:1], None, mybir.AluOpType.is_equal
        )
        # e8T[j, p] = (j == p//16) on partitions 0..7 (bf16, lhsT of the
        # alpha/beta selection matmul)
        prow = cpool.tile([8, 128], F32, tag="prow")
        nc.gpsimd.iota(
            prow[:], pattern=[[1, 128]], base=0, channel_multiplier=0,
            allow_small_or_imprecise_dtypes=True,
        )
        jcol8 = cpool.tile([8, 1], F32, tag="jcol8")
        nc.gpsimd.iota(
            jcol8[:], pattern=[[0, 1]], base=0, channel_multiplier=16,
            allow_small_or_imprecise_dtypes=True,
        )
        e8T = cpool.tile([8, 128], BF16, tag="e8T")
        # e8T[j, p] = (p - 16j) in [0, 16): build via (p//16 == j) using
        # shifted compare: is_equal(prow*1/16 floor?) -> use range compare:
        # (prow - 16j) in [0,16)  ==  (prow >= 16j) * (prow < 16j+16)
        ge_t = cpool.tile([8, 128], F32, tag="ge_t")
        nc.vector.tensor_scalar(
            ge_t[:], prow[:], jcol8[:, 0:1], None,
            mybir.AluOpType.is_ge,
        )
        lt_t = cpool.tile([8, 128], F32, tag="lt_t")
        jcol8b = cpool.tile([8, 1], F32, tag="jcol8b")
        nc.vector.tensor_scalar(
            jcol8b[:], jcol8[:], 16.0, None, mybir.AluOpType.add
        )
        nc.vector.tensor_scalar(
            lt_t[:], prow[:], jcol8b[:, 0:1], None, mybir.AluOpType.is_lt
        )
        nc.vector.tensor_tensor(e8T[:], ge_t[:], lt_t[:], op=mybir.AluOpType.mult)
        ones_bf = cpool.tile([128, 1], BF16, tag="ones_bf")
        nc.vector.memset(ones_bf[:], 1.0)
        ones_f = cpool.tile([128, 1], F32, tag="ones_f")
        nc.vector.memset(ones_f[:], 1.0)
        warm_sb = cpool.tile([1, 8], F32, tag="warm_sb")
        nc.vector.memset(warm_sb[:], 1.0)

        # ---------- big tiles ----------
        xt = cpool.tile([128, FB * B], BF16, tag="xt")  # [p, fb*B + ci*128 + s]
        xsq = cpool.tile([128, FB * B], BF16, tag="xsq")
        ssT = spool.tile([128, FB * C], F32, tag="ssT")  # class sums [f, fb*C + c]
        ctab = spool.tile([128, FB * NCH * 16], BF16, tag="ctab")  # [f, fb, ci, hj]
        sqc = spool.tile([128, FB * C], BF16, tag="sqc")
        sqr = spool.tile([128, FB * C], BF16, tag="sqr")
        abrow = spool.tile([1, NCH * 16], BF16, tag="abrow")
        tparts = spool.tile([128, FB * NG], F32, tag="tparts")
        tvec = spool.tile([128, FB], F32, tag="tvec")
        tcol_s = spool.tile([128, FB], F32, tag="tcol_s")
        scol = spool.tile([128, 2 * NCH], F32, tag="scol")  # [p, ci*2 + h]
        out_sb = spool.tile([1, 1], F32, tag="out_sb")

        # ---------- PSUM ----------
        qps = qpool.tile([128, NCH], F32, tag="qps")
        gps = [
            gpool.tile([128, 512], F32, tag=f"gps{b}", name=f"gps{b}")
            for b in range(2)
        ]
        absel_ps = ppool2.tile([128, 2 * NCH], F32, tag="absel")
        aps = ppool2.tile([1, C], F32, tag="aps")
        bps = ppool2.tile([1, C], F32, tag="bps")

        xt3 = xt[:].rearrange("p (fb b) -> p fb b", fb=FB)
        xt_dr3 = xt_dram.ap().rearrange("p (fb b) -> p fb b", fb=FB)
        xsq3 = xsq[:].rearrange("p (fb b) -> p fb b", fb=FB)
        ssT3 = ssT[:].rearrange("p (fb c) -> p fb c", fb=FB)
        ctab4 = ctab[:].rearrange(
            "p (fb ci hj) -> p fb ci hj", fb=FB, ci=NCH
        )
        tparts3 = tparts[:].rearrange("p (fb g) -> p fb g", fb=FB)

        # ---------- front: issue all input DMAs, then the warmup AR ------
        for g in range(NG):
            csl = slice(g * GC * 128, (g + 1) * GC * 128)
            eng = nc.sync if g % 2 == 0 else nc.gpsimd
            eng.dma_start(xt3[:, :, csl], xt_dr3[:, :, csl])
        # warmup collective absorbs cross-core launch skew; issued after the
        # gpsimd DMA triggers so it doesn't stall them
        warm_in = drpool.tile([1, 8], F32, name="warm_in")
        warm_out = drpool.tile([1, 8], F32, name="warm_out")
        nc.sync.dma_start(warm_in[:], warm_sb[:])
        if stage >= 5:
            nc.gpsimd.collective_compute(
                "AllReduce",
                mybir.AluOpType.add,
                replica_groups=[list(range(n_cores))],
                ins=[warm_in.opt()],
                outs=[warm_out.opt()],
            )

        # ---------- front: per group of 8 chunks ----------
        for g in range(NG):
            csl = slice(g * GC * 128, (g + 1) * GC * 128)
            # squares (ACT mostly, last group on DVE to unclog ACT)
            if g < NG - 1:
                nc.scalar.activation(
                    xsq3[:, :, csl], xt3[:, :, csl],
                    mybir.ActivationFunctionType.Square,
                )
            else:
                nc.vector.tensor_tensor(
                    xsq3[:, :, csl], xt3[:, :, csl], xt3[:, :, csl],
                    op=mybir.AluOpType.mult,
                )
            # segmented class sums: [128, fb, 64 classes, 16 samples] -> sum
            gin = bass.AP(
                xt3.tensor,
                xt3.offset + g * GC * 128,
                [xt3.ap[0], [B, FB], [16, 64], [1, 16]],
            )
            nc.vector.tensor_reduce(
                ssT3[:, :, g * 64 : (g + 1) * 64],
                gin,
                op=mybir.AluOpType.add,
                axis=mybir.AxisListType.X,
            )
            # running total T (per-group partial: sum of this group's classes)
            nc.vector.tensor_reduce(
                tparts3[:, :, g : g + 1],
                ssT3[:, :, g * 64 : (g + 1) * 64],
                op=mybir.AluOpType.add,
                axis=mybir.AxisListType.X,
            )
            # q matmuls: qps[:, ci] = sum_f xsq[f, ci-chunk]
            for k in range(GC):
                ci = g * GC + k
                for fb in range(FB):
                    nc.tensor.matmul(
                        qps[:, ci : ci + 1],
                        lhsT=xsq3[:, fb, ci * 128 : (ci + 1) * 128],
                        rhs=ones_bf[:, 0:1],
                        start=(fb == 0),
                        stop=(fb == 1),
                    )

        # ---------- mid: tables ----------
        nc.vector.tensor_reduce(
            tvec[:],
            tparts3,
            op=mybir.AluOpType.add,
            axis=mybir.AxisListType.X,
        )
        nc.vector.tensor_scalar(
            tcol_s[:], tvec[:], 2.0 / ((C - 1) * K), None, mybir.AluOpType.mult
        )
        # cm = -2*cent = ssT * (-2/K)   -> ctab[., ., ., 0:8]   (ACT)
        nc.scalar.mul(ctab4[:, :, :, 0:8], ssT3, -2.0 / K)
        # crm = -2*rest = ssT*(-2/((C-1)K)) + T*(2/((C-1)K)) -> [., 8:16] (DVE)
        for fb in range(FB):
            nc.vector.tensor_scalar(
                ctab4[:, fb, :, 8:16],
                ssT3[:, fb, :],
                -2.0 / ((C - 1) * K),
                tcol_s[:, fb : fb + 1],
                mybir.AluOpType.mult,
                mybir.AluOpType.add,
            )
        # squared tables for |cent|^2, |rest|^2
        nc.scalar.activation(
            sqc[:].rearrange("p (fb c) -> p fb c", fb=FB),
            ctab4[:, :, :, 0:8],
            mybir.ActivationFunctionType.Square,
        )
        nc.vector.tensor_tensor(
            sqr[:].rearrange("p (fb c) -> p fb c", fb=FB),
            ctab4[:, :, :, 8:16],
            ctab4[:, :, :, 8:16],
            op=mybir.AluOpType.mult,
        )
        for fb in range(FB):
            nc.tensor.matmul(
                aps[:],
                lhsT=ones_bf[:, 0:1],
                rhs=sqc[:, fb * C : (fb + 1) * C],
                start=(fb == 0),
                stop=(fb == 1),
            )
            nc.tensor.matmul(
                bps[:],
                lhsT=ones_bf[:, 0:1],
                rhs=sqr[:, fb * C : (fb + 1) * C],
                start=(fb == 0),
                stop=(fb == 1),
            )
        # abrow[0, ci*16 + h*8 + j] = (h ? beta : alpha)[8ci + j] = psum/4
        ab4 = abrow[:].rearrange("o (ci h j) -> o ci h j", ci=NCH, h=2)
        nc.scalar.mul(ab4[:, :, 0, :], aps[:], 0.25)
        nc.vector.tensor_scalar(
            ab4[:, :, 1, :], bps[:], 0.25, None, mybir.AluOpType.mult
        )
        # round-trip through DRAM to repartition into ab2[j, ci*2+h]
        ab_dr = drpool.tile([1, NCH * 16], BF16, name="ab_dr")
        nc.sync.dma_start(ab_dr[:], abrow[:])
        ab2 = spool.tile([8, 2 * NCH], BF16, tag="ab2")
        ab_dr_r = bass.AP(
            ab_dr.tensor, ab_dr.offset, [[1, 8], [16, NCH], [8, 2]]
        )
        nc.sync.dma_start(ab2[:], ab_dr_r)
        # absel_ps[p, ci*2+h] = (h ? beta : alpha)[class(p, ci)]
        nc.tensor.matmul(
            absel_ps[:], lhsT=e8T[:], rhs=ab2[:], start=True, stop=True
        )

        cc_out1 = cc_out2 = None
        if stage >= 3:
            # ---------- dots + selection, half-slab at a time ----------
            for b in range(2):
                for k in range(32):
                    ci = b * 32 + k
                    for fb in range(FB):
                        nc.tensor.matmul(
                            gps[b][:, 16 * k : 16 * k + 16],
                            lhsT=xt3[:, fb, ci * 128 : (ci + 1) * 128],
                            rhs=ctab4[:, fb, ci, :],
                            start=(fb == 0),
                            stop=(fb == 1),
                        )
                # masked select: red[p, k, h] = sum_j gps[p,k,h,j]*mask8[p,j]
                gv = gps[b][:].rearrange("p (k h j) -> p k h j", k=32, h=2)
                m8 = mask8[:]
                mb = bass.AP(
                    m8.tensor, m8.offset,
                    [m8.ap[0], [0, 32], [0, 2], [1, 8]],
                )
                msk = spool.tile([128, 512], F32, tag=f"msk{b}", name=f"msk{b}")
                msk4 = msk[:].rearrange("p (k h j) -> p k h j", k=32, h=2)
                nc.vector.tensor_tensor(msk4, gv, mb, op=mybir.AluOpType.mult)
                red = spool.tile([128, 64], F32, tag=f"red{b}", name=f"red{b}")
                nc.vector.tensor_reduce(
                    red[:].rearrange("p (k h) -> p k h", k=32),
                    msk4,
                    op=mybir.AluOpType.add,
                    axis=mybir.AxisListType.X,
                )
                # scol[:, b*64 : b*64+64] = red + absel + q (q broadcast on h)
                nc.vector.tensor_tensor(
                    red[:], red[:], absel_ps[:, b * 64 : (b + 1) * 64],
                    op=mybir.AluOpType.add,
                )
                qv = qps[:]
                qb = bass.AP(
                    qv.tensor, qv.offset + b * 32,
                    [qv.ap[0], [1, 32], [0, 2]],
                )
                nc.vector.tensor_tensor(
                    scol[:, b * 64 : (b + 1) * 64], red[:], qb,
                    op=mybir.AluOpType.add,
                )
                if stage >= 4 and b == 0:
                    cc_out1 = _ar(
                        nc, drpool, scol[:, 0:NCH], n_cores, stage, "h1"
                    )

        if stage >= 4:
            cc_out2 = _ar(
                nc, drpool, scol[:, NCH : 2 * NCH], n_cores, stage, "h2"
            )
            # ---------- tail ----------
            sres = spool.tile([128, 2 * NCH], F32, tag="sres")
            nc.sync.dma_start(sres[:, 0:NCH], cc_out1[:])
            nc.sync.dma_start(sres[:, NCH : 2 * NCH], cc_out2[:])
            s3 = sres[:].rearrange("p (ci h) -> p ci h", h=2)
            nc.scalar.dma_start(
                dbg_dram.ap()[:, 0:NCH], s3[:, :, 0:1]
            )
            nc.scalar.dma_start(
                dbg_dram.ap()[:, NCH : 2 * NCH], s3[:, :, 1:2]
            )
            rt = spool.tile([128, 2 * NCH], F32, tag="rt")
            nc.scalar.sqrt(rt[:], sres[:])
            rt3 = rt[:].rearrange("p (ci h) -> p ci h", h=2)
            num = spool.tile([128, NCH], F32, tag="num")
            den = spool.tile([128, NCH], F32, tag="den")
            nc.vector.tensor_tensor(
                num[:], s3[:, :, 0:1], s3[:, :, 1:2],
                op=mybir.AluOpType.subtract,
            )
            nc.vector.tensor_tensor(
                den[:], rt3[:, :, 0:1], rt3[:, :, 1:2],
                op=mybir.AluOpType.add,
            )
            rden = spool.tile([128, NCH], F32, tag="rden")
            nc.vector.reciprocal(rden[:], den[:])
            delta = spool.tile([128, NCH], F32, tag="delta")
            nc.vector.tensor_tensor(
                delta[:], num[:], rden[:], op=mybir.AluOpType.mult
            )
            terms = spool.tile([128, NCH], F32, tag="terms")
            lcol = spool.tile([128, 1], F32, tag="lcol")
            margin_t = spool.tile([128, 1], F32, tag="margin")
            nc.vector.memset(margin_t[:], MARGIN)
            nc.scalar.activation(
                terms[:],
                delta[:],
                mybir.ActivationFunctionType.Relu,
                bias=margin_t[:, 0:1],
                scale=1.0,
                accum_out=lcol[:, 0:1],
            )
            loss_ps = ppool2.tile([1, 1], F32, tag="loss")
            nc.tensor.matmul(
                loss_ps[:], lhsT=ones_f[:, 0:1], rhs=lcol[:, 0:1],
                start=True, stop=True,
            )
            nc.scalar.mul(out_sb[:], loss_ps[:], 1.0 / B)
        else:
            nc.scalar.mul(out_sb[:], scol[0:1, 0:1], 1.0)
            s3d = scol[:].rearrange("p (ci h) -> p ci h", h=2)
            nc.sync.dma_start(dbg_dram.ap()[:, 0:NCH], s3d[:, :, 0:1])
            nc.sync.dma_start(dbg_dram.ap()[:, NCH : 2 * NCH], s3d[:, :, 1:2])

        nc.sync.dma_start(loss_dram.ap(), out_sb[:])

    nc.compile()
    return nc


def build_general(stage=5, n_cores=N_CORES):
    """Correct for arbitrary targets in [0, C)."""
    nc = bacc.Bacc(None, target_bir_lowering=False, debug=False, num_devices=n_cores)
    x_dram = nc.dram_tensor("x", [B, DS], F32, kind="ExternalInput")
    tgt_dram = nc.dram_tensor("tgt", [128, NCH], I32, kind="ExternalInput")
    loss_dram = nc.dram_tensor("loss", [1, 1], F32, kind="ExternalOutput")
    dbg_dram = nc.dram_tensor("dbg", [128, 2 * NCH], F32, kind="ExternalOutput")

    with tile.TileContext(nc) as tc, ExitStack() as top:
        cpool = top.enter_context(tc.tile_pool(name="const", bufs=1))
        ohpool = top.enter_context(tc.tile_pool(name="oh", bufs=4))
        gpool = top.enter_context(tc.tile_pool(name="gath", bufs=4))
        dpool = top.enter_context(tc.tile_pool(name="diff", bufs=3))
        spool = top.enter_context(tc.tile_pool(name="small", bufs=1))
        ppool2 = top.enter_context(tc.tile_pool(name="psum2", bufs=1, space="PSUM"))
        drpool = top.enter_context(tc.tile_pool(name="dram", bufs=1, space="DRAM"))

        xres = cpool.tile([128, NCH * CW], F32, tag="xres")
        x3 = xres[:].rearrange("p (c w) -> p c w", w=CW)
        nc.vector.memset(x3[:, :, DS : DS + 1], 1.0)
        iota_t = cpool.tile([128, C], F32, tag="iota")
        nc.gpsimd.iota(
            iota_t[:], pattern=[[1, C]], base=0, channel_multiplier=0,
            allow_small_or_imprecise_dtypes=True,
        )
        tg32 = cpool.tile([128, NCH], I32, tag="tg32")
        nc.sync.dma_start(tg32[:], tgt_dram.ap())
        tgf = cpool.tile([128, NCH], F32, tag="tgf")
        nc.vector.tensor_copy(tgf[:], tg32[:])
        ones_col = cpool.tile([128, 1], F32, tag="ones_col")
        nc.vector.memset(ones_col[:], 1.0)
        ones_row = cpool.tile([1, 128], F32, tag="ones_row")
        nc.vector.memset(ones_row[:], 1.0)

        xr = x_dram.ap().rearrange("(c p) d -> p c d", p=128)
        for g in range(8):
            nc.sync.dma_start(
                x3[:, g * 8 : (g + 1) * 8, 0:DS], xr[:, g * 8 : (g + 1) * 8, :]
            )

        scol = spool.tile([128, 2 * NCH], F32, tag="scol")
        out_sb = spool.tile([1, 1], F32, tag="out_sb")

        with ExitStack() as ph1:
            ppool1 = ph1.enter_context(
                tc.tile_pool(name="psum1", bufs=1, space="PSUM")
            )
            sums_ps = [
                ppool1.tile([128, CW], F32, tag=f"sums{g}", name=f"sums{g}")
                for g in range(CG)
            ]
            for ci in range(NCH):
                a_t = ohpool.tile([128, C], F32, tag="onehot")
                nc.vector.tensor_scalar(
                    a_t[:],
                    iota_t[:],
                    tgf[:, ci : ci + 1],
                    None,
                    mybir.AluOpType.is_equal,
                )
                rhs = xres[:, ci * CW : (ci + 1) * CW]
                for g in range(CG):
                    nc.tensor.matmul(
                        sums_ps[g][:],
                        lhsT=a_t[:, g * 128 : (g + 1) * 128],
                        rhs=rhs,
                        start=(ci == 0),
                        stop=(ci == NCH - 1),
                    )

            cent = [
                spool.tile([128, DS], F32, tag=f"cent{g}", name=f"cent{g}")
                for g in range(CG)
            ]
            rest = [
                spool.tile([128, DS], F32, tag=f"rest{g}", name=f"rest{g}")
                for g in range(CG)
            ]
            recip = [
                spool.tile([128, 1], F32, tag=f"recip{g}", name=f"recip{g}")
                for g in range(CG)
            ]
            for g in range(CG):
                nc.vector.reciprocal(recip[g][:], sums_ps[g][:, DS : DS + 1])
                nc.vector.tensor_scalar(
                    cent[g][:],
                    sums_ps[g][:, 0:DS],
                    recip[g][:, 0:1],
                    None,
                    mybir.AluOpType.mult,
                )
            tot_ps = ppool1.tile([1, DS], F32, tag="tot")
            for g in range(CG):
                nc.tensor.matmul(
                    tot_ps[:],
                    lhsT=ones_col[:, 0:1],
                    rhs=cent[g][:],
                    start=(g == 0),
                    stop=(g == CG - 1),
                )
            tot_sb = spool.tile([1, DS], F32, tag="tot_sb")
            nc.scalar.mul(tot_sb[:], tot_ps[:], 1.0 / (C - 1))
            tb_ps = ppool1.tile([128, DS], F32, tag="tb")
            nc.tensor.matmul(
                tb_ps[:], lhsT=ones_row[:], rhs=tot_sb[:], start=True, stop=True
            )
            resttmp = spool.tile([128, DS], F32, tag="resttmp")
            for g in range(CG):
                nc.vector.tensor_scalar_mul(resttmp[:], cent[g][:], 1.0 / (C - 1))
                nc.vector.tensor_tensor(
                    rest[g][:], tb_ps[:], resttmp[:], op=mybir.AluOpType.subtract
                )
            table = drpool.tile([C, 2 * DS], F32)
            for g in range(CG):
                nc.sync.dma_start(table[g * 128 : (g + 1) * 128, 0:DS], cent[g][:])
                nc.sync.dma_start(
                    table[g * 128 : (g + 1) * 128, DS : 2 * DS], rest[g][:]
                )

        if stage >= 3:
            for ci in range(NCH):
                cg_t = gpool.tile([128, 2 * DS], F32, tag="gath")
                nc.gpsimd.indirect_dma_start(
                    out=cg_t[:],
                    out_offset=None,
                    in_=table[:],
                    in_offset=IndirectOffsetOnAxis(ap=tg32[:, ci : ci + 1], axis=0),
                )
                xch = xres[:, ci * CW : ci * CW + DS]
                dap = dpool.tile([128, DS], F32, tag="dap")
                dan = dpool.tile([128, DS], F32, tag="dan")
                nc.vector.tensor_tensor(
                    dap[:], xch, cg_t[:, 0:DS], op=mybir.AluOpType.subtract
                )
                nc.vector.tensor_tensor(
                    dan[:], xch, cg_t[:, DS : 2 * DS], op=mybir.AluOpType.subtract
                )
                nc.scalar.activation(
                    dap[:],
                    dap[:],
                    mybir.ActivationFunctionType.Square,
                    accum_out=scol[:, 2 * ci : 2 * ci + 1],
                )
                nc.scalar.activation(
                    dan[:],
                    dan[:],
                    mybir.ActivationFunctionType.Square,
                    accum_out=scol[:, 2 * ci + 1 : 2 * ci + 2],
                )

        if stage >= 4:
            cc_out1 = _ar(nc, drpool, scol[:, 0:NCH], n_cores, stage, "h1")
            cc_out2 = _ar(
                nc, drpool, scol[:, NCH : 2 * NCH], n_cores, stage, "h2"
            )
            _loss_tail(
                nc, spool, ppool2, drpool, dbg_dram, out_sb, cc_out1, cc_out2,
                stage,
            )
        else:
            nc.scalar.mul(out_sb[:], scol[0:1, 0:1], 1.0)
            nc.sync.dma_start(dbg_dram.ap()[:, 0:NCH], scol[:, 0:NCH])
            nc.sync.dma_start(
                dbg_dram.ap()[:, NCH : 2 * NCH], scol[:, NCH : 2 * NCH]
            )

        nc.sync.dma_start(loss_dram.ap(), out_sb[:])

    nc.compile()
    return nc


_PROGRAMS = {}


def _get_program(sorted_fast):
    if sorted_fast not in _PROGRAMS:
        _PROGRAMS[sorted_fast] = (
            build_sorted() if sorted_fast else build_general()
        )
    return _PROGRAMS[sorted_fast]


def is_sorted_balanced(t):
    return bool(np.array_equal(t, np.arange(B, dtype=np.int64) // K))


def make_in_maps(inputs, targets, sorted_fast):
    import ml_dtypes

    x = np.asarray(inputs, dtype=np.float32)
    assert x.shape == (B, D), x.shape
    if sorted_fast:
        # feature-major bf16: xt[p, fb*B + b] = x[b, c*DS + fb*128 + p]
        xm = x.astype(ml_dtypes.bfloat16)
        maps = []
        for c in range(N_CORES):
            xs = xm[:, c * DS : (c + 1) * DS].T  # [DS, B]
            xs = np.ascontiguousarray(
                xs.reshape(2, 128, B).transpose(1, 0, 2).reshape(128, 2 * B)
            )
            maps.append({"xt": xs})
        return maps
    t = np.asarray(targets).astype(np.int32)
    tgt_re = np.ascontiguousarray(t.reshape(NCH, 128).T)  # [128, NCH]
    return [
        {
            "x": np.ascontiguousarray(x[:, c * DS : (c + 1) * DS]),
            "tgt": tgt_re,
        }
        for c in range(N_CORES)
    ]


def kernel(inputs, targets, num_classes, **_unused):
    assert int(num_classes) == C
    sf = is_sorted_balanced(np.asarray(targets))
    nc = _get_program(sf)
    in_maps = make_in_maps(inputs, targets, sf)
    res = run_bass_kernel_spmd(nc, in_maps, core_ids=list(range(N_CORES)))
    val = np.float32(res.results[0]["loss"][0, 0])
    return np.asarray(val, dtype=np.float32).reshape(())



# revision 16
# speedup vs baseline: 2.8279x; 2.8279x over previous
"""CentroidTripletLoss Trainium2 kernel (8 NeuronCores, feature-dim sharded).

Math (matches the reference):
    centroids[c] = mean of inputs with target c           (segment mean)
    rest[c]      = (sum_c' centroids[c'] - centroids[c]) / (C-1)
    d_ap[b] = ||x_b - centroids[t_b]||,  d_an[b] = ||x_b - rest[t_b]||
    loss = mean(relu(d_ap - d_an + MARGIN))

Distribution: the feature dim D=2048 is sharded 8 ways (256 per core).
Each core computes complete per-class sums for its feature slice (no
centroid all-reduce needed), then per-sample partial squared distances;
a single 64KB AllReduce combines the partials, after which every core
finishes the (tiny) scalar loss reduction redundantly.

Two compiled variants share this builder:
  * sorted_fast: targets are exactly arange(B)//(B//C) (the identity-
    balanced sampler in the reference).  The per-chunk one-hot matrices
    are then compile-time constants (16 distinct patterns), the segment
    sum is one bf16 matmul per chunk, and the per-sample [centroid|rest]
    rows are produced by TensorE from a chunk-major SBUF table
    (diff = E @ [cent|rest] + I @ [-x|-x], accumulated in PSUM), so no
    gather DMA exists at all.
  * general: any targets in [0, C).  fp32 one-hot x 4 class-group
    matmuls, row gather via indirect DMA, VectorE subtracts.
The host picks the variant per call, so arbitrary inputs stay correct.

d_ap - d_an is evaluated as (sap - san) / (sqrt(sap) + sqrt(san)) so the
loose HW sqrt (large ULP budget) only perturbs the result by its own
relative error instead of being amplified by cancellation.
"""

from contextlib import ExitStack

import numpy as np

import concourse.bacc as bacc
import concourse.bass as bass
import concourse.tile as tile
from concourse import mybir
from concourse.bass import IndirectOffsetOnAxis
from concourse.bass_utils import run_bass_kernel_spmd

N_CORES = 8
B = 8192
D = 2048
DS = D // N_CORES  # 256 features per core
C = 512
K = B // C  # 16 samples per class when identity-balanced
NCH = B // 128  # 64 chunks of 128 samples
CG = C // 128  # 4 class groups
CW = DS + 1  # chunk width in resident fp32 X tile (features + ones col)
MARGIN = 0.3

F32 = mybir.dt.float32
BF16 = mybir.dt.bfloat16
I32 = mybir.dt.int32


def _ar(nc, drpool, src_ap, n_cores, stage, name):
    """AllReduce a [128, 64] f32 slab; returns the output DRAM tile."""
    cc_in = drpool.tile([128, NCH], F32, name=f"cc_in_{name}")
    cc_out = drpool.tile([128, NCH], F32, name=f"cc_out_{name}")
    nc.sync.dma_start(cc_in[:], src_ap)
    if stage >= 5:
        nc.gpsimd.collective_compute(
            "AllReduce",
            mybir.AluOpType.add,
            replica_groups=[list(range(n_cores))],
            ins=[cc_in.opt()],
            outs=[cc_out.opt()],
        )
    else:
        nc.sync.dma_start(cc_out[:], cc_in[:])
    return cc_out


def _loss_tail(nc, spool, ppool2, drpool, dbg_dram, out_sb, cc_out1, cc_out2,
               stage):
    """Finish the scalar loss from the two AllReduced interleaved slabs."""
    ones_f = spool.tile([128, 1], F32, tag="ones_f")
    nc.vector.memset(ones_f[:], 1.0)
    sres = spool.tile([128, 2 * NCH], F32, tag="sres")
    nc.sync.dma_start(sres[:, 0:NCH], cc_out1[:])
    nc.sync.dma_start(sres[:, NCH : 2 * NCH], cc_out2[:])
    s3 = sres[:].rearrange("p (c two) -> p c two", two=2)
    sapg = spool.tile([128, NCH], F32, tag="sapg")
    sang = spool.tile([128, NCH], F32, tag="sang")
    nc.vector.tensor_copy(sapg[:], s3[:, :, 0:1])
    nc.vector.tensor_copy(sang[:], s3[:, :, 1:2])
    nc.scalar.dma_start(dbg_dram.ap()[:, 0:NCH], sapg[:])
    nc.scalar.dma_start(dbg_dram.ap()[:, NCH : 2 * NCH], sang[:])
    dapf = spool.tile([128, NCH], F32, tag="dapf")
    danf = spool.tile([128, NCH], F32, tag="danf")
    nc.scalar.sqrt(dapf[:], sapg[:])
    nc.scalar.sqrt(danf[:], sang[:])
    num = spool.tile([128, NCH], F32, tag="num")
    den = spool.tile([128, NCH], F32, tag="den")
    nc.vector.tensor_tensor(num[:], sapg[:], sang[:], op=mybir.AluOpType.subtract)
    nc.vector.tensor_tensor(den[:], dapf[:], danf[:], op=mybir.AluOpType.add)
    rden = spool.tile([128, NCH], F32, tag="rden")
    nc.vector.reciprocal(rden[:], den[:])
    delta = spool.tile([128, NCH], F32, tag="delta")
    nc.vector.tensor_tensor(delta[:], num[:], rden[:], op=mybir.AluOpType.mult)
    terms = spool.tile([128, NCH], F32, tag="terms")
    lcol = spool.tile([128, 1], F32, tag="lcol")
    margin_t = spool.tile([128, 1], F32, tag="margin")
    nc.vector.memset(margin_t[:], MARGIN)
    nc.scalar.activation(
        terms[:],
        delta[:],
        mybir.ActivationFunctionType.Relu,
        bias=margin_t[:, 0:1],
        scale=1.0,
        accum_out=lcol[:, 0:1],
    )
    loss_ps = ppool2.tile([1, 1], F32, tag="loss")
    nc.tensor.matmul(
        loss_ps[:], lhsT=ones_f[:, 0:1], rhs=lcol[:, 0:1], start=True, stop=True
    )
    nc.scalar.mul(out_sb[:], loss_ps[:], 1.0 / B)


def build_sorted(stage=5, n_cores=N_CORES):
    """Fast path: targets == arange(B)//K (verified on host).

    Algebraic form: with q=|x|^2, cent=S/K, rest=(T-cent)/(C-1):
        sap = q - 2 x.cent[t] + |cent[t]|^2
        san = q - 2 x.rest[t] + |rest[t]|^2
    Ships X feature-major (xt[f, b]); centroids come from a segmented DVE
    reduce (16 consecutive samples per class), q from per-chunk matmuls of
    squared xt against ones, the dots from per-chunk matmuls against a
    [-2cent | -2rest] table, class selection via a masked DVE reduce, and
    the per-class norm constants via a tiny E8 matmul. All pieces are
    linear in the feature shard, so one fp32 AllReduce of [128, 128]
    (split in 2 to overlap) combines the 8 cores.
    """
    nc = bacc.Bacc(None, target_bir_lowering=False, debug=False, num_devices=n_cores)
    NG = 8  # DMA/compute groups (8 chunks each)
    GC = NCH // NG  # 8 chunks per group
    FB = 2  # feature blocks of 128
    xt_dram = nc.dram_tensor("xt", [128, FB * B], BF16, kind="ExternalInput")
    loss_dram = nc.dram_tensor("loss", [1, 1], F32, kind="ExternalOutput")
    dbg_dram = nc.dram_tensor("dbg", [128, 2 * NCH], F32, kind="ExternalOutput")

    with tile.TileContext(nc) as tc, ExitStack() as top:
        cpool = top.enter_context(tc.tile_pool(name="const", bufs=1))
        spool = top.enter_context(tc.tile_pool(name="small", bufs=1))
        qpool = top.enter_context(tc.tile_pool(name="qps", bufs=1, space="PSUM"))
        gpool = top.enter_context(tc.tile_pool(name="gps", bufs=1, space="PSUM"))
        ppool2 = top.enter_context(tc.tile_pool(name="psum2", bufs=1, space="PSUM"))
        drpool = top.enter_context(tc.tile_pool(name="dram", bufs=1, space="DRAM"))

        # ---------- constants ----------
        pcol_i = cpool.tile([128, 1], I32, tag="pcol_i")
        nc.gpsimd.iota(pcol_i[:], pattern=[[0, 1]], base=0, channel_multiplier=1)
        p16_i = cpool.tile([128, 1], I32, tag="p16_i")
        nc.vector.tensor_scalar(
            p16_i[:], pcol_i[:], 4, None, mybir.AluOpType.arith_shift_right
        )
        p16_f = cpool.tile([128, 1], F32, tag="p16_f")
        nc.vector.tensor_copy(p16_f[:], p16_i[:])
        jrow8 = cpool.tile([128, 8], F32, tag="jrow8")
        nc.gpsimd.iota(
            jrow8[:], pattern=[[1, 8]], base=0, channel_multiplier=0,
            allow_small_or_imprecise_dtypes=True,
        )
        # mask8[p, j] = (j == p//16), fp32 for the masked PSUM reduce
        mask8 = cpool.tile([128, 8], F32, tag="mask8")
        nc.vector.tensor_scalar(
            mask8[:], jrow8[:], p16_f[:, 0:1], None, mybir.AluOpType.is_equal
        )
        # e8T[j, p] = (j == p//16) on partitions 0..7 (bf16, lhsT of the
        # alpha/beta selection matmul)
        prow = cpool.tile([8, 128], F32, tag="prow")
        nc.gpsimd.iota(
            prow[:], pattern=[[1, 128]], base=0, channel_multiplier=0,
            allow_small_or_imprecise_dtypes=True,
        )
        jcol8 = cpool.tile([8, 1], F32, tag="jcol8")
        nc.gpsimd.iota(
            jcol8[:], pattern=[[0, 1]], base=0, channel_multiplier=16,
            allow_small_or_imprecise_dtypes=True,
        )
        e8T = cpool.tile([8, 128], BF16, tag="e8T")
        # e8T[j, p] = (p - 16j) in [0, 16): build via (p//16 == j) using
        # shifted compare: is_equal(prow*1/16 floor?) -> use range compare:
        # (prow - 16j) in [0,16)  ==  (prow >= 16j) * (prow < 16j+16)
        ge_t = cpool.tile([8, 128], F32, tag="ge_t")
        nc.vector.tensor_scalar(
            ge_t[:], prow[:], jcol8[:, 0:1], None,
            mybir.AluOpType.is_ge,
        )
        lt_t = cpool.tile([8, 128], F32, tag="lt_t")
        jcol8b = cpool.tile([8, 1], F32, tag="jcol8b")
        nc.vector.tensor_scalar(
            jcol8b[:], jcol8[:], 16.0, None, mybir.AluOpType.add
        )
        nc.vector.tensor_scalar(
            lt_t[:], prow[:], jcol8b[:, 0:1], None, mybir.AluOpType.is_lt
        )
        nc.vector.tensor_tensor(e8T[:], ge_t[:], lt_t[:], op=mybir.AluOpType.mult)
        ones_bf = cpool.tile([128, 1], BF16, tag="ones_bf")
        nc.vector.memset(ones_bf[:], 1.0)
        ones_f = cpool.tile([128, 1], F32, tag="ones_f")
        nc.vector.memset(ones_f[:], 1.0)
        warm_sb = cpool.tile([1, 8], F32, tag="warm_sb")
        nc.vector.memset(warm_sb[:], 1.0)

        # ---------- big tiles ----------
        xt = cpool.tile([128, FB * B], BF16, tag="xt")  # [p, fb*B + ci*128 + s]
        xsq = cpool.tile([128, FB * B], BF16, tag="xsq")
        ssT = spool.tile([128, FB * C], F32, tag="ssT")  # class sums [f, fb*C + c]
        ctab = spool.tile([128, FB * NCH * 16], BF16, tag="ctab")  # [f, fb, ci, hj]
        sqc = spool.tile([128, FB * C], BF16, tag="sqc")
        sqr = spool.tile([128, FB * C], BF16, tag="sqr")
        abrow = spool.tile([1, NCH * 16], BF16, tag="abrow")
        tparts = spool.tile([128, FB * NG], F32, tag="tparts")
        tvec = spool.tile([128, FB], F32, tag="tvec")
        tcol_s = spool.tile([128, FB], F32, tag="tcol_s")
        scol = spool.tile([128, 2 * NCH], F32, tag="scol")  # [p, ci*2 + h]
        out_sb = spool.tile([1, 1], F32, tag="out_sb")

        # ---------- PSUM ----------
        qps = qpool.tile([128, NCH], F32, tag="qps")
        gps = [
            gpool.tile([128, 512], F32, tag=f"gps{b}", name=f"gps{b}")
            for b in range(2)
        ]
        absel_ps = ppool2.tile([128, 2 * NCH], F32, tag="absel")
        aps = ppool2.tile([1, C], F32, tag="aps")
        bps = ppool2.tile([1, C], F32, tag="bps")

        xt3 = xt[:].rearrange("p (fb b) -> p fb b", fb=FB)
        xt_dr3 = xt_dram.ap().rearrange("p (fb b) -> p fb b", fb=FB)
        xsq3 = xsq[:].rearrange("p (fb b) -> p fb b", fb=FB)
        ssT3 = ssT[:].rearrange("p (fb c) -> p fb c", fb=FB)
        ctab4 = ctab[:].rearrange(
            "p (fb ci hj) -> p fb ci hj", fb=FB, ci=NCH
        )
        tparts3 = tparts[:].rearrange("p (fb g) -> p fb g", fb=FB)

        # ---------- front: issue all input DMAs, then the warmup AR ------
        # contiguous [128, 2048] slabs (4KB/partition runs), fb-interleaved
        # so chunk quarter k is complete after slab pair k
        for k in range(4):
            for fb in range(FB):
                sl = slice(fb * B + k * 2048, fb * B + (k + 1) * 2048)
                eng = nc.sync if fb == 0 else nc.gpsimd
                eng.dma_start(xt[:, sl], xt_dram.ap()[:, sl])
        # warmup collective absorbs cross-core launch skew; issued after the
        # gpsimd DMA triggers so it doesn't stall them
        warm_in = drpool.tile([1, 8], F32, name="warm_in")
        warm_out = drpool.tile([1, 8], F32, name="warm_out")
        nc.sync.dma_start(warm_in[:], warm_sb[:])
        if stage >= 5:
            nc.gpsimd.collective_compute(
                "AllReduce",
                mybir.AluOpType.add,
                replica_groups=[list(range(n_cores))],
                ins=[warm_in.opt()],
                outs=[warm_out.opt()],
            )

        # ---------- front: per group of 8 chunks ----------
        for g in range(NG):
            csl = slice(g * GC * 128, (g + 1) * GC * 128)
            # squares (ACT mostly, last group on DVE to unclog ACT)
            if g < NG - 1:
                nc.scalar.activation(
                    xsq3[:, :, csl], xt3[:, :, csl],
                    mybir.ActivationFunctionType.Square,
                )
            else:
                nc.vector.tensor_tensor(
                    xsq3[:, :, csl], xt3[:, :, csl], xt3[:, :, csl],
                    op=mybir.AluOpType.mult,
                )
            # segmented class sums: [128, fb, 64 classes, 16 samples] -> sum
            gin = bass.AP(
                xt3.tensor,
                xt3.offset + g * GC * 128,
                [xt3.ap[0], [B, FB], [16, 64], [1, 16]],
            )
            nc.vector.tensor_reduce(
                ssT3[:, :, g * 64 : (g + 1) * 64],
                gin,
                op=mybir.AluOpType.add,
                axis=mybir.AxisListType.X,
            )
            # running total T (per-group partial: sum of this group's classes)
            nc.vector.tensor_reduce(
                tparts3[:, :, g : g + 1],
                ssT3[:, :, g * 64 : (g + 1) * 64],
                op=mybir.AluOpType.add,
                axis=mybir.AxisListType.X,
            )
            # q matmuls: qps[:, ci] = sum_f xsq[f, ci-chunk]
            for k in range(GC):
                ci = g * GC + k
                for fb in range(FB):
                    nc.tensor.matmul(
                        qps[:, ci : ci + 1],
                        lhsT=xsq3[:, fb, ci * 128 : (ci + 1) * 128],
                        rhs=ones_bf[:, 0:1],
                        start=(fb == 0),
                        stop=(fb == 1),
                    )

        # ---------- mid: tables ----------
        nc.vector.tensor_reduce(
            tvec[:],
            tparts3,
            op=mybir.AluOpType.add,
            axis=mybir.AxisListType.X,
        )
        nc.vector.tensor_scalar(
            tcol_s[:], tvec[:], -2.0 / ((C - 1) * K), None, mybir.AluOpType.mult
        )
        # cm = -2*cent = ssT * (-2/K)   -> ctab[., ., ., 0:8]   (ACT)
        nc.scalar.mul(ctab4[:, :, :, 0:8], ssT3, -2.0 / K)
        # crm = -2*rest = -2*(T - cent)/(C-1)
        #     = ssT*(+2/((C-1)K)) + T*(-2/((C-1)K))  -> [., 8:16] (DVE)
        for fb in range(FB):
            nc.vector.tensor_scalar(
                ctab4[:, fb, :, 8:16],
                ssT3[:, fb, :],
                2.0 / ((C - 1) * K),
                tcol_s[:, fb : fb + 1],
                mybir.AluOpType.mult,
                mybir.AluOpType.add,
            )
        # squared tables for |cent|^2, |rest|^2
        nc.scalar.activation(
            sqc[:].rearrange("p (fb c) -> p fb c", fb=FB),
            ctab4[:, :, :, 0:8],
            mybir.ActivationFunctionType.Square,
        )
        nc.vector.tensor_tensor(
            sqr[:].rearrange("p (fb c) -> p fb c", fb=FB),
            ctab4[:, :, :, 8:16],
            ctab4[:, :, :, 8:16],
            op=mybir.AluOpType.mult,
        )
        for fb in range(FB):
            nc.tensor.matmul(
                aps[:],
                lhsT=ones_bf[:, 0:1],
                rhs=sqc[:, fb * C : (fb + 1) * C],
                start=(fb == 0),
                stop=(fb == 1),
            )
            nc.tensor.matmul(
                bps[:],
                lhsT=ones_bf[:, 0:1],
                rhs=sqr[:, fb * C : (fb + 1) * C],
                start=(fb == 0),
                stop=(fb == 1),
            )
        # abrow[0, ci*16 + h*8 + j] = (h ? beta : alpha)[8ci + j] = psum/4
        ab4 = abrow[:].rearrange("o (ci h j) -> o ci h j", ci=NCH, h=2)
        nc.scalar.mul(ab4[:, :, 0, :], aps[:], 0.25)
        nc.vector.tensor_scalar(
            ab4[:, :, 1, :], bps[:], 0.25, None, mybir.AluOpType.mult
        )
        # round-trip through DRAM to repartition into ab2[j, ci*2+h]
        ab_dr = drpool.tile([1, NCH * 16], BF16, name="ab_dr")
        nc.sync.dma_start(ab_dr[:], abrow[:])
        ab2 = spool.tile([8, 2 * NCH], BF16, tag="ab2")
        ab_dr_r = bass.AP(
            ab_dr.tensor, ab_dr.offset, [[1, 8], [16, NCH], [8, 2]]
        )
        nc.sync.dma_start(ab2[:], ab_dr_r)
        # absel_ps[p, ci*2+h] = (h ? beta : alpha)[class(p, ci)]
        nc.tensor.matmul(
            absel_ps[:], lhsT=e8T[:], rhs=ab2[:], start=True, stop=True
        )

        cc_out1 = cc_out2 = None
        if stage >= 3:
            # ---------- dots + selection, half-slab at a time ----------
            for b in range(2):
                for k in range(32):
                    ci = b * 32 + k
                    for fb in range(FB):
                        nc.tensor.matmul(
                            gps[b][:, 16 * k : 16 * k + 16],
                            lhsT=xt3[:, fb, ci * 128 : (ci + 1) * 128],
                            rhs=ctab4[:, fb, ci, :],
                            start=(fb == 0),
                            stop=(fb == 1),
                        )
                # masked select: red[p, k, h] = sum_j gps[p,k,h,j]*mask8[p,j]
                gv = gps[b][:].rearrange("p (k h j) -> p k h j", k=32, h=2)
                m8 = mask8[:]
                mb = bass.AP(
                    m8.tensor, m8.offset,
                    [m8.ap[0], [0, 32], [0, 2], [1, 8]],
                )
                msk = spool.tile([128, 512], F32, tag=f"msk{b}", name=f"msk{b}")
                msk4 = msk[:].rearrange("p (k h j) -> p k h j", k=32, h=2)
                nc.vector.tensor_tensor(msk4, gv, mb, op=mybir.AluOpType.mult)
                red = spool.tile([128, 64], F32, tag=f"red{b}", name=f"red{b}")
                nc.vector.tensor_reduce(
                    red[:].rearrange("p (k h) -> p k h", k=32),
                    msk4,
                    op=mybir.AluOpType.add,
                    axis=mybir.AxisListType.X,
                )
                # scol[:, b*64 : b*64+64] = red + absel + q (q broadcast on h)
                nc.vector.tensor_tensor(
                    red[:], red[:], absel_ps[:, b * 64 : (b + 1) * 64],
                    op=mybir.AluOpType.add,
                )
                qv = qps[:]
                qb = bass.AP(
                    qv.tensor, qv.offset + b * 32,
                    [qv.ap[0], [1, 32], [0, 2]],
                )
                nc.vector.tensor_tensor(
                    scol[:, b * 64 : (b + 1) * 64], red[:], qb,
                    op=mybir.AluOpType.add,
                )
                if stage >= 4 and b == 0:
                    cc_out1 = _ar(
                        nc, drpool, scol[:, 0:NCH], n_cores, stage, "h1"
                    )

        if stage >= 4:
            cc_out2 = _ar(
                nc, drpool, scol[:, NCH : 2 * NCH], n_cores, stage, "h2"
            )
            # ---------- tail ----------
            sres = spool.tile([128, 2 * NCH], F32, tag="sres")
            nc.sync.dma_start(sres[:, 0:NCH], cc_out1[:])
            nc.sync.dma_start(sres[:, NCH : 2 * NCH], cc_out2[:])
            s3 = sres[:].rearrange("p (ci h) -> p ci h", h=2)
            # dbg ships interleaved [p, ci*2+h]; host de-interleaves
            nc.sync.dma_start(dbg_dram.ap(), sres[:])
            rt = spool.tile([128, 2 * NCH], F32, tag="rt")
            nc.scalar.sqrt(rt[:], sres[:])
            rt3 = rt[:].rearrange("p (ci h) -> p ci h", h=2)
            num = spool.tile([128, NCH], F32, tag="num")
            den = spool.tile([128, NCH], F32, tag="den")
            nc.vector.tensor_tensor(
                num[:], s3[:, :, 0:1], s3[:, :, 1:2],
                op=mybir.AluOpType.subtract,
            )
            nc.vector.tensor_tensor(
                den[:], rt3[:, :, 0:1], rt3[:, :, 1:2],
                op=mybir.AluOpType.add,
            )
            rden = spool.tile([128, NCH], F32, tag="rden")
            nc.vector.reciprocal(rden[:], den[:])
            delta = spool.tile([128, NCH], F32, tag="delta")
            nc.vector.tensor_tensor(
                delta[:], num[:], rden[:], op=mybir.AluOpType.mult
            )
            terms = spool.tile([128, NCH], F32, tag="terms")
            lcol = spool.tile([128, 1], F32, tag="lcol")
            margin_t = spool.tile([128, 1], F32, tag="margin")
            nc.vector.memset(margin_t[:], MARGIN)
            nc.scalar.activation(
                terms[:],
                delta[:],
                mybir.ActivationFunctionType.Relu,
                bias=margin_t[:, 0:1],
                scale=1.0,
                accum_out=lcol[:, 0:1],
            )
            loss_ps = ppool2.tile([1, 1], F32, tag="loss")
            nc.tensor.matmul(
                loss_ps[:], lhsT=ones_f[:, 0:1], rhs=lcol[:, 0:1],
                start=True, stop=True,
            )
            nc.scalar.mul(out_sb[:], loss_ps[:], 1.0 / B)
        else:
            nc.scalar.mul(out_sb[:], scol[0:1, 0:1], 1.0)
            nc.sync.dma_start(dbg_dram.ap(), scol[:])

        nc.sync.dma_start(loss_dram.ap(), out_sb[:])

    nc.compile()
    return nc


def build_general(stage=5, n_cores=N_CORES):
    """Correct for arbitrary targets in [0, C)."""
    nc = bacc.Bacc(None, target_bir_lowering=False, debug=False, num_devices=n_cores)
    x_dram = nc.dram_tensor("x", [B, DS], F32, kind="ExternalInput")
    tgt_dram = nc.dram_tensor("tgt", [128, NCH], I32, kind="ExternalInput")
    loss_dram = nc.dram_tensor("loss", [1, 1], F32, kind="ExternalOutput")
    dbg_dram = nc.dram_tensor("dbg", [128, 2 * NCH], F32, kind="ExternalOutput")

    with tile.TileContext(nc) as tc, ExitStack() as top:
        cpool = top.enter_context(tc.tile_pool(name="const", bufs=1))
        ohpool = top.enter_context(tc.tile_pool(name="oh", bufs=4))
        gpool = top.enter_context(tc.tile_pool(name="gath", bufs=4))
        dpool = top.enter_context(tc.tile_pool(name="diff", bufs=3))
        spool = top.enter_context(tc.tile_pool(name="small", bufs=1))
        ppool2 = top.enter_context(tc.tile_pool(name="psum2", bufs=1, space="PSUM"))
        drpool = top.enter_context(tc.tile_pool(name="dram", bufs=1, space="DRAM"))

        xres = cpool.tile([128, NCH * CW], F32, tag="xres")
        x3 = xres[:].rearrange("p (c w) -> p c w", w=CW)
        nc.vector.memset(x3[:, :, DS : DS + 1], 1.0)
        iota_t = cpool.tile([128, C], F32, tag="iota")
        nc.gpsimd.iota(
            iota_t[:], pattern=[[1, C]], base=0, channel_multiplier=0,
            allow_small_or_imprecise_dtypes=True,
        )
        tg32 = cpool.tile([128, NCH], I32, tag="tg32")
        nc.sync.dma_start(tg32[:], tgt_dram.ap())
        tgf = cpool.tile([128, NCH], F32, tag="tgf")
        nc.vector.tensor_copy(tgf[:], tg32[:])
        ones_col = cpool.tile([128, 1], F32, tag="ones_col")
        nc.vector.memset(ones_col[:], 1.0)
        ones_row = cpool.tile([1, 128], F32, tag="ones_row")
        nc.vector.memset(ones_row[:], 1.0)

        xr = x_dram.ap().rearrange("(c p) d -> p c d", p=128)
        for g in range(8):
            nc.sync.dma_start(
                x3[:, g * 8 : (g + 1) * 8, 0:DS], xr[:, g * 8 : (g + 1) * 8, :]
            )

        scol = spool.tile([128, 2 * NCH], F32, tag="scol")
        out_sb = spool.tile([1, 1], F32, tag="out_sb")

        with ExitStack() as ph1:
            ppool1 = ph1.enter_context(
                tc.tile_pool(name="psum1", bufs=1, space="PSUM")
            )
            sums_ps = [
                ppool1.tile([128, CW], F32, tag=f"sums{g}", name=f"sums{g}")
                for g in range(CG)
            ]
            for ci in range(NCH):
                a_t = ohpool.tile([128, C], F32, tag="onehot")
                nc.vector.tensor_scalar(
                    a_t[:],
                    iota_t[:],
                    tgf[:, ci : ci + 1],
                    None,
                    mybir.AluOpType.is_equal,
                )
                rhs = xres[:, ci * CW : (ci + 1) * CW]
                for g in range(CG):
                    nc.tensor.matmul(
                        sums_ps[g][:],
                        lhsT=a_t[:, g * 128 : (g + 1) * 128],
                        rhs=rhs,
                        start=(ci == 0),
                        stop=(ci == NCH - 1),
                    )

            cent = [
                spool.tile([128, DS], F32, tag=f"cent{g}", name=f"cent{g}")
                for g in range(CG)
            ]
            rest = [
                spool.tile([128, DS], F32, tag=f"rest{g}", name=f"rest{g}")
                for g in range(CG)
            ]
            recip = [
                spool.tile([128, 1], F32, tag=f"recip{g}", name=f"recip{g}")
                for g in range(CG)
            ]
            for g in range(CG):
                nc.vector.reciprocal(recip[g][:], sums_ps[g][:, DS : DS + 1])
                nc.vector.tensor_scalar(
                    cent[g][:],
                    sums_ps[g][:, 0:DS],
                    recip[g][:, 0:1],
                    None,
                    mybir.AluOpType.mult,
                )
            tot_ps = ppool1.tile([1, DS], F32, tag="tot")
            for g in range(CG):
                nc.tensor.matmul(
                    tot_ps[:],
                    lhsT=ones_col[:, 0:1],
                    rhs=cent[g][:],
                    start=(g == 0),
                    stop=(g == CG - 1),
                )
            tot_sb = spool.tile([1, DS], F32, tag="tot_sb")
            nc.scalar.mul(tot_sb[:], tot_ps[:], 1.0 / (C - 1))
            tb_ps = ppool1.tile([128, DS], F32, tag="tb")
            nc.tensor.matmul(
                tb_ps[:], lhsT=ones_row[:], rhs=tot_sb[:], start=True, stop=True
            )
            resttmp = spool.tile([128, DS], F32, tag="resttmp")
            for g in range(CG):
                nc.vector.tensor_scalar_mul(resttmp[:], cent[g][:], 1.0 / (C - 1))
                nc.vector.tensor_tensor(
                    rest[g][:], tb_ps[:], resttmp[:], op=mybir.AluOpType.subtract
                )
            table = drpool.tile([C, 2 * DS], F32)
            for g in range(CG):
                nc.sync.dma_start(table[g * 128 : (g + 1) * 128, 0:DS], cent[g][:])
                nc.sync.dma_start(
                    table[g * 128 : (g + 1) * 128, DS : 2 * DS], rest[g][:]
                )

        if stage >= 3:
            for ci in range(NCH):
                cg_t = gpool.tile([128, 2 * DS], F32, tag="gath")
                nc.gpsimd.indirect_dma_start(
                    out=cg_t[:],
                    out_offset=None,
                    in_=table[:],
                    in_offset=IndirectOffsetOnAxis(ap=tg32[:, ci : ci + 1], axis=0),
                )
                xch = xres[:, ci * CW : ci * CW + DS]
                dap = dpool.tile([128, DS], F32, tag="dap")
                dan = dpool.tile([128, DS], F32, tag="dan")
                nc.vector.tensor_tensor(
                    dap[:], xch, cg_t[:, 0:DS], op=mybir.AluOpType.subtract
                )
                nc.vector.tensor_tensor(
                    dan[:], xch, cg_t[:, DS : 2 * DS], op=mybir.AluOpType.subtract
                )
                nc.scalar.activation(
                    dap[:],
                    dap[:],
                    mybir.ActivationFunctionType.Square,
                    accum_out=scol[:, 2 * ci : 2 * ci + 1],
                )
                nc.scalar.activation(
                    dan[:],
                    dan[:],
                    mybir.ActivationFunctionType.Square,
                    accum_out=scol[:, 2 * ci + 1 : 2 * ci + 2],
                )

        if stage >= 4:
            cc_out1 = _ar(nc, drpool, scol[:, 0:NCH], n_cores, stage, "h1")
            cc_out2 = _ar(
                nc, drpool, scol[:, NCH : 2 * NCH], n_cores, stage, "h2"
            )
            _loss_tail(
                nc, spool, ppool2, drpool, dbg_dram, out_sb, cc_out1, cc_out2,
                stage,
            )
        else:
            nc.scalar.mul(out_sb[:], scol[0:1, 0:1], 1.0)
            nc.sync.dma_start(dbg_dram.ap()[:, 0:NCH], scol[:, 0:NCH])
            nc.sync.dma_start(
                dbg_dram.ap()[:, NCH : 2 * NCH], scol[:, NCH : 2 * NCH]
            )

        nc.sync.dma_start(loss_dram.ap(), out_sb[:])

    nc.compile()
    return nc


_PROGRAMS = {}


def _get_program(sorted_fast):
    if sorted_fast not in _PROGRAMS:
        _PROGRAMS[sorted_fast] = (
            build_sorted() if sorted_fast else build_general()
        )
    return _PROGRAMS[sorted_fast]


def is_sorted_balanced(t):
    return bool(np.array_equal(t, np.arange(B, dtype=np.int64) // K))


def make_in_maps(inputs, targets, sorted_fast):
    import ml_dtypes

    x = np.asarray(inputs, dtype=np.float32)
    assert x.shape == (B, D), x.shape
    if sorted_fast:
        # feature-major bf16: xt[p, fb*B + b] = x[b, c*DS + fb*128 + p]
        xm = x.astype(ml_dtypes.bfloat16)
        maps = []
        for c in range(N_CORES):
            xs = xm[:, c * DS : (c + 1) * DS].T  # [DS, B]
            xs = np.ascontiguousarray(
                xs.reshape(2, 128, B).transpose(1, 0, 2).reshape(128, 2 * B)
            )
            maps.append({"xt": xs})
        return maps
    t = np.asarray(targets).astype(np.int32)
    tgt_re = np.ascontiguousarray(t.reshape(NCH, 128).T)  # [128, NCH]
    return [
        {
            "x": np.ascontiguousarray(x[:, c * DS : (c + 1) * DS]),
            "tgt": tgt_re,
        }
        for c in range(N_CORES)
    ]


def kernel(inputs, targets, num_classes, **_unused):
    assert int(num_classes) == C
    sf = is_sorted_balanced(np.asarray(targets))
    nc = _get_program(sf)
    in_maps = make_in_maps(inputs, targets, sf)
    res = run_bass_kernel_spmd(nc, in_maps, core_ids=list(range(N_CORES)))
    val = np.float32(res.results[0]["loss"][0, 0])
    return np.asarray(val, dtype=np.float32).reshape(())



# revision 19
# speedup vs baseline: 2.9635x; 1.0479x over previous
"""CentroidTripletLoss Trainium2 kernel (8 NeuronCores, feature-dim sharded).

Math (matches the reference):
    centroids[c] = mean of inputs with target c           (segment mean)
    rest[c]      = (sum_c' centroids[c'] - centroids[c]) / (C-1)
    d_ap[b] = ||x_b - centroids[t_b]||,  d_an[b] = ||x_b - rest[t_b]||
    loss = mean(relu(d_ap - d_an + MARGIN))

Distribution: the feature dim D=2048 is sharded 8 ways (256 per core).
Each core computes complete per-class sums for its feature slice (no
centroid all-reduce needed), then per-sample partial squared distances;
a single 64KB AllReduce combines the partials, after which every core
finishes the (tiny) scalar loss reduction redundantly.

Two compiled variants share this builder:
  * sorted_fast: targets are exactly arange(B)//(B//C) (the identity-
    balanced sampler in the reference).  The per-chunk one-hot matrices
    are then compile-time constants (16 distinct patterns), the segment
    sum is one bf16 matmul per chunk, and the per-sample [centroid|rest]
    rows are produced by TensorE from a chunk-major SBUF table
    (diff = E @ [cent|rest] + I @ [-x|-x], accumulated in PSUM), so no
    gather DMA exists at all.
  * general: any targets in [0, C).  fp32 one-hot x 4 class-group
    matmuls, row gather via indirect DMA, VectorE subtracts.
The host picks the variant per call, so arbitrary inputs stay correct.

d_ap - d_an is evaluated as (sap - san) / (sqrt(sap) + sqrt(san)) so the
loose HW sqrt (large ULP budget) only perturbs the result by its own
relative error instead of being amplified by cancellation.
"""

from contextlib import ExitStack

import numpy as np

import concourse.bacc as bacc
import concourse.bass as bass
import concourse.tile as tile
from concourse import mybir
from concourse.bass import IndirectOffsetOnAxis
from concourse.bass_utils import run_bass_kernel_spmd

N_CORES = 8
B = 8192
D = 2048
DS = D // N_CORES  # 256 features per core
C = 512
K = B // C  # 16 samples per class when identity-balanced
NCH = B // 128  # 64 chunks of 128 samples
CG = C // 128  # 4 class groups
CW = DS + 1  # chunk width in resident fp32 X tile (features + ones col)
MARGIN = 0.3

F32 = mybir.dt.float32
BF16 = mybir.dt.bfloat16
I32 = mybir.dt.int32


def _ar(nc, drpool, src_ap, n_cores, stage, name):
    """AllReduce a [128, 64] f32 slab; returns the output DRAM tile."""
    cc_in = drpool.tile([128, NCH], F32, name=f"cc_in_{name}")
    cc_out = drpool.tile([128, NCH], F32, name=f"cc_out_{name}")
    nc.sync.dma_start(cc_in[:], src_ap)
    if stage >= 5:
        nc.gpsimd.collective_compute(
            "AllReduce",
            mybir.AluOpType.add,
            replica_groups=[list(range(n_cores))],
            ins=[cc_in.opt()],
            outs=[cc_out.opt()],
        )
    else:
        nc.sync.dma_start(cc_out[:], cc_in[:])
    return cc_out


def _loss_tail(nc, spool, ppool2, drpool, dbg_dram, out_sb, cc_out1, cc_out2,
               stage):
    """Finish the scalar loss from the two AllReduced interleaved slabs."""
    ones_f = spool.tile([128, 1], F32, tag="ones_f")
    nc.vector.memset(ones_f[:], 1.0)
    sres = spool.tile([128, 2 * NCH], F32, tag="sres")
    nc.sync.dma_start(sres[:, 0:NCH], cc_out1[:])
    nc.sync.dma_start(sres[:, NCH : 2 * NCH], cc_out2[:])
    s3 = sres[:].rearrange("p (c two) -> p c two", two=2)
    sapg = spool.tile([128, NCH], F32, tag="sapg")
    sang = spool.tile([128, NCH], F32, tag="sang")
    nc.vector.tensor_copy(sapg[:], s3[:, :, 0:1])
    nc.vector.tensor_copy(sang[:], s3[:, :, 1:2])
    nc.scalar.dma_start(dbg_dram.ap()[:, 0:NCH], sapg[:])
    nc.scalar.dma_start(dbg_dram.ap()[:, NCH : 2 * NCH], sang[:])
    dapf = spool.tile([128, NCH], F32, tag="dapf")
    danf = spool.tile([128, NCH], F32, tag="danf")
    nc.scalar.sqrt(dapf[:], sapg[:])
    nc.scalar.sqrt(danf[:], sang[:])
    num = spool.tile([128, NCH], F32, tag="num")
    den = spool.tile([128, NCH], F32, tag="den")
    nc.vector.tensor_tensor(num[:], sapg[:], sang[:], op=mybir.AluOpType.subtract)
    nc.vector.tensor_tensor(den[:], dapf[:], danf[:], op=mybir.AluOpType.add)
    rden = spool.tile([128, NCH], F32, tag="rden")
    nc.vector.reciprocal(rden[:], den[:])
    delta = spool.tile([128, NCH], F32, tag="delta")
    nc.vector.tensor_tensor(delta[:], num[:], rden[:], op=mybir.AluOpType.mult)
    terms = spool.tile([128, NCH], F32, tag="terms")
    lcol = spool.tile([128, 1], F32, tag="lcol")
    margin_t = spool.tile([128, 1], F32, tag="margin")
    nc.vector.memset(margin_t[:], MARGIN)
    nc.scalar.activation(
        terms[:],
        delta[:],
        mybir.ActivationFunctionType.Relu,
        bias=margin_t[:, 0:1],
        scale=1.0,
        accum_out=lcol[:, 0:1],
    )
    loss_ps = ppool2.tile([1, 1], F32, tag="loss")
    nc.tensor.matmul(
        loss_ps[:], lhsT=ones_f[:, 0:1], rhs=lcol[:, 0:1], start=True, stop=True
    )
    nc.scalar.mul(out_sb[:], loss_ps[:], 1.0 / B)


def build_sorted(stage=5, n_cores=N_CORES):
    """Fast path: targets == arange(B)//K (verified on host).

    Algebraic form: with q=|x|^2, cent=S/K, rest=(T-cent)/(C-1):
        sap = q - 2 x.cent[t] + |cent[t]|^2
        san = q - 2 x.rest[t] + |rest[t]|^2
    Ships X feature-major (xt[f, b]); centroids come from a segmented DVE
    reduce (16 consecutive samples per class), q from per-chunk matmuls of
    squared xt against ones, the dots from per-chunk matmuls against a
    [-2cent | -2rest] table, class selection via a masked DVE reduce, and
    the per-class norm constants via a tiny E8 matmul. All pieces are
    linear in the feature shard, so one fp32 AllReduce of [128, 128]
    (split in 2 to overlap) combines the 8 cores.
    """
    nc = bacc.Bacc(None, target_bir_lowering=False, debug=False, num_devices=n_cores)
    NG = 8  # DMA/compute groups (8 chunks each)
    GC = NCH // NG  # 8 chunks per group
    FB = 2  # feature blocks of 128
    xt_dram = nc.dram_tensor("xt", [128, FB * B], BF16, kind="ExternalInput")
    loss_dram = nc.dram_tensor("loss", [1, 1], F32, kind="ExternalOutput")
    dbg_dram = nc.dram_tensor("dbg", [128, 2 * NCH], F32, kind="ExternalOutput")

    with tile.TileContext(nc) as tc, ExitStack() as top:
        cpool = top.enter_context(tc.tile_pool(name="const", bufs=1))
        spool = top.enter_context(tc.tile_pool(name="small", bufs=1))
        qpool = top.enter_context(tc.tile_pool(name="qps", bufs=1, space="PSUM"))
        gpool = top.enter_context(tc.tile_pool(name="gps", bufs=1, space="PSUM"))
        ppool2 = top.enter_context(tc.tile_pool(name="psum2", bufs=1, space="PSUM"))
        drpool = top.enter_context(tc.tile_pool(name="dram", bufs=1, space="DRAM"))

        # ---------- constants ----------
        pcol_i = cpool.tile([128, 1], I32, tag="pcol_i")
        nc.gpsimd.iota(pcol_i[:], pattern=[[0, 1]], base=0, channel_multiplier=1)
        p16_i = cpool.tile([128, 1], I32, tag="p16_i")
        nc.vector.tensor_scalar(
            p16_i[:], pcol_i[:], 4, None, mybir.AluOpType.arith_shift_right
        )
        p16_f = cpool.tile([128, 1], F32, tag="p16_f")
        nc.vector.tensor_copy(p16_f[:], p16_i[:])
        jrow8 = cpool.tile([128, 8], F32, tag="jrow8")
        nc.gpsimd.iota(
            jrow8[:], pattern=[[1, 8]], base=0, channel_multiplier=0,
            allow_small_or_imprecise_dtypes=True,
        )
        # mask8[p, j] = (j == p//16), fp32 for the masked PSUM reduce
        mask8 = cpool.tile([128, 8], F32, tag="mask8")
        nc.vector.tensor_scalar(
            mask8[:], jrow8[:], p16_f[:, 0:1], None, mybir.AluOpType.is_equal
        )
        # e8T[j, p] = (j == p//16) on partitions 0..7 (bf16, lhsT of the
        # alpha/beta selection matmul)
        prow = cpool.tile([8, 128], F32, tag="prow")
        nc.gpsimd.iota(
            prow[:], pattern=[[1, 128]], base=0, channel_multiplier=0,
            allow_small_or_imprecise_dtypes=True,
        )
        jcol8 = cpool.tile([8, 1], F32, tag="jcol8")
        nc.gpsimd.iota(
            jcol8[:], pattern=[[0, 1]], base=0, channel_multiplier=16,
            allow_small_or_imprecise_dtypes=True,
        )
        e8T = cpool.tile([8, 128], BF16, tag="e8T")
        # e8T[j, p] = (p - 16j) in [0, 16): build via (p//16 == j) using
        # shifted compare: is_equal(prow*1/16 floor?) -> use range compare:
        # (prow - 16j) in [0,16)  ==  (prow >= 16j) * (prow < 16j+16)
        ge_t = cpool.tile([8, 128], F32, tag="ge_t")
        nc.vector.tensor_scalar(
            ge_t[:], prow[:], jcol8[:, 0:1], None,
            mybir.AluOpType.is_ge,
        )
        lt_t = cpool.tile([8, 128], F32, tag="lt_t")
        jcol8b = cpool.tile([8, 1], F32, tag="jcol8b")
        nc.vector.tensor_scalar(
            jcol8b[:], jcol8[:], 16.0, None, mybir.AluOpType.add
        )
        nc.vector.tensor_scalar(
            lt_t[:], prow[:], jcol8b[:, 0:1], None, mybir.AluOpType.is_lt
        )
        nc.vector.tensor_tensor(e8T[:], ge_t[:], lt_t[:], op=mybir.AluOpType.mult)
        ones_bf = cpool.tile([128, 1], BF16, tag="ones_bf")
        nc.vector.memset(ones_bf[:], 1.0)
        ones_f = cpool.tile([128, 1], F32, tag="ones_f")
        nc.vector.memset(ones_f[:], 1.0)
        warm_sb = cpool.tile([1, 8], F32, tag="warm_sb")
        nc.vector.memset(warm_sb[:], 1.0)

        # ---------- big tiles ----------
        xt = cpool.tile([128, FB * B], BF16, tag="xt")  # [p, fb*B + ci*128 + s]
        xsq = cpool.tile([128, FB * B], BF16, tag="xsq")
        ssT = spool.tile([128, FB * C], F32, tag="ssT")  # class sums [f, fb*C + c]
        ctab = spool.tile([128, FB * NCH * 16], BF16, tag="ctab")  # [f, fb, ci, hj]
        sqc = spool.tile([128, FB * C], BF16, tag="sqc")
        sqr = spool.tile([128, FB * C], BF16, tag="sqr")
        abrow = spool.tile([1, NCH * 16], BF16, tag="abrow")
        tparts = spool.tile([128, FB * NG], F32, tag="tparts")
        tvec = spool.tile([128, FB], F32, tag="tvec")
        tcol_s = spool.tile([128, FB], F32, tag="tcol_s")
        scol = spool.tile([128, 2 * NCH], F32, tag="scol")  # [p, ci*2 + h]
        out_sb = spool.tile([1, 1], F32, tag="out_sb")

        # ---------- PSUM ----------
        qps = qpool.tile([128, NCH], F32, tag="qps")
        gps = [
            gpool.tile([128, 512], F32, tag=f"gps{b}", name=f"gps{b}")
            for b in range(2)
        ]
        absel_ps = ppool2.tile([128, 2 * NCH], F32, tag="absel")
        aps = ppool2.tile([1, C], F32, tag="aps")
        bps = ppool2.tile([1, C], F32, tag="bps")

        xt3 = xt[:].rearrange("p (fb b) -> p fb b", fb=FB)
        xt_dr3 = xt_dram.ap().rearrange("p (fb b) -> p fb b", fb=FB)
        xsq3 = xsq[:].rearrange("p (fb b) -> p fb b", fb=FB)
        ssT3 = ssT[:].rearrange("p (fb c) -> p fb c", fb=FB)
        ctab4 = ctab[:].rearrange(
            "p (fb ci hj) -> p fb ci hj", fb=FB, ci=NCH
        )
        tparts3 = tparts[:].rearrange("p (fb g) -> p fb g", fb=FB)

        # ---------- front: issue all input DMAs over 4 trigger queues ----
        # contiguous [128, 2048] slabs (4KB/partition runs), fb-interleaved
        # so chunk quarter k is complete after slab pair k
        dma_engs = [nc.sync, nc.gpsimd, nc.scalar]
        for k in range(4):
            for fb in range(FB):
                sl = slice(fb * B + k * 2048, fb * B + (k + 1) * 2048)
                dma_engs[(2 * k + fb) % 3].dma_start(
                    xt[:, sl], xt_dram.ap()[:, sl]
                )

        # ---------- front: per group of 8 chunks ----------
        for g in range(NG):
            csl = slice(g * GC * 128, (g + 1) * GC * 128)
            # squares (ACT mostly, last group on DVE to unclog ACT)
            if g < NG - 1:
                nc.scalar.activation(
                    xsq3[:, :, csl], xt3[:, :, csl],
                    mybir.ActivationFunctionType.Square,
                )
            else:
                nc.vector.tensor_tensor(
                    xsq3[:, :, csl], xt3[:, :, csl], xt3[:, :, csl],
                    op=mybir.AluOpType.mult,
                )
            # segmented class sums: [128, fb, 64 classes, 16 samples] -> sum
            gin = bass.AP(
                xt3.tensor,
                xt3.offset + g * GC * 128,
                [xt3.ap[0], [B, FB], [16, 64], [1, 16]],
            )
            nc.vector.tensor_reduce(
                ssT3[:, :, g * 64 : (g + 1) * 64],
                gin,
                op=mybir.AluOpType.add,
                axis=mybir.AxisListType.X,
            )
            # running total T (per-group partial: sum of this group's classes)
            nc.vector.tensor_reduce(
                tparts3[:, :, g : g + 1],
                ssT3[:, :, g * 64 : (g + 1) * 64],
                op=mybir.AluOpType.add,
                axis=mybir.AxisListType.X,
            )
            # q matmuls: qps[:, ci] = sum_f xsq[f, ci-chunk]
            for k in range(GC):
                ci = g * GC + k
                for fb in range(FB):
                    nc.tensor.matmul(
                        qps[:, ci : ci + 1],
                        lhsT=xsq3[:, fb, ci * 128 : (ci + 1) * 128],
                        rhs=ones_bf[:, 0:1],
                        start=(fb == 0),
                        stop=(fb == 1),
                    )

        # ---------- mid: tables ----------
        nc.vector.tensor_reduce(
            tvec[:],
            tparts3,
            op=mybir.AluOpType.add,
            axis=mybir.AxisListType.X,
        )
        nc.vector.tensor_scalar(
            tcol_s[:], tvec[:], -2.0 / ((C - 1) * K), None, mybir.AluOpType.mult
        )
        # cm = -2*cent = ssT * (-2/K)   -> ctab[., ., ., 0:8]   (ACT)
        nc.scalar.mul(ctab4[:, :, :, 0:8], ssT3, -2.0 / K)
        # crm = -2*rest = -2*(T - cent)/(C-1)
        #     = ssT*(+2/((C-1)K)) + T*(-2/((C-1)K))  -> [., 8:16] (DVE)
        for fb in range(FB):
            nc.vector.tensor_scalar(
                ctab4[:, fb, :, 8:16],
                ssT3[:, fb, :],
                2.0 / ((C - 1) * K),
                tcol_s[:, fb : fb + 1],
                mybir.AluOpType.mult,
                mybir.AluOpType.add,
            )
        # squared tables for |cent|^2, |rest|^2
        nc.scalar.activation(
            sqc[:].rearrange("p (fb c) -> p fb c", fb=FB),
            ctab4[:, :, :, 0:8],
            mybir.ActivationFunctionType.Square,
        )
        nc.vector.tensor_tensor(
            sqr[:].rearrange("p (fb c) -> p fb c", fb=FB),
            ctab4[:, :, :, 8:16],
            ctab4[:, :, :, 8:16],
            op=mybir.AluOpType.mult,
        )
        for fb in range(FB):
            nc.tensor.matmul(
                aps[:],
                lhsT=ones_bf[:, 0:1],
                rhs=sqc[:, fb * C : (fb + 1) * C],
                start=(fb == 0),
                stop=(fb == 1),
            )
            nc.tensor.matmul(
                bps[:],
                lhsT=ones_bf[:, 0:1],
                rhs=sqr[:, fb * C : (fb + 1) * C],
                start=(fb == 0),
                stop=(fb == 1),
            )
        # abrow[0, ci*16 + h*8 + j] = (h ? beta : alpha)[8ci + j] = psum/4
        ab4 = abrow[:].rearrange("o (ci h j) -> o ci h j", ci=NCH, h=2)
        nc.scalar.mul(ab4[:, :, 0, :], aps[:], 0.25)
        nc.vector.tensor_scalar(
            ab4[:, :, 1, :], bps[:], 0.25, None, mybir.AluOpType.mult
        )
        # round-trip through DRAM to repartition into ab2[j, ci*2+h]
        ab_dr = drpool.tile([1, NCH * 16], BF16, name="ab_dr")
        nc.sync.dma_start(ab_dr[:], abrow[:])
        ab2 = spool.tile([8, 2 * NCH], BF16, tag="ab2")
        ab_dr_r = bass.AP(
            ab_dr.tensor, ab_dr.offset, [[1, 8], [16, NCH], [8, 2]]
        )
        nc.sync.dma_start(ab2[:], ab_dr_r)
        # absel_ps[p, ci*2+h] = (h ? beta : alpha)[class(p, ci)]
        nc.tensor.matmul(
            absel_ps[:], lhsT=e8T[:], rhs=ab2[:], start=True, stop=True
        )

        cc_out1 = cc_out2 = None
        if stage >= 3:
            # ---------- dots + selection, half-slab at a time ----------
            for b in range(2):
                for k in range(32):
                    ci = b * 32 + k
                    for fb in range(FB):
                        nc.tensor.matmul(
                            gps[b][:, 16 * k : 16 * k + 16],
                            lhsT=xt3[:, fb, ci * 128 : (ci + 1) * 128],
                            rhs=ctab4[:, fb, ci, :],
                            start=(fb == 0),
                            stop=(fb == 1),
                        )
                # masked select: red[p, k, h] = sum_j gps[p,k,h,j]*mask8[p,j]
                gv = gps[b][:].rearrange("p (k h j) -> p k h j", k=32, h=2)
                m8 = mask8[:]
                mb = bass.AP(
                    m8.tensor, m8.offset,
                    [m8.ap[0], [0, 32], [0, 2], [1, 8]],
                )
                msk = spool.tile([128, 512], F32, tag=f"msk{b}", name=f"msk{b}")
                msk4 = msk[:].rearrange("p (k h j) -> p k h j", k=32, h=2)
                nc.vector.tensor_tensor(msk4, gv, mb, op=mybir.AluOpType.mult)
                red = spool.tile([128, 64], F32, tag=f"red{b}", name=f"red{b}")
                nc.vector.tensor_reduce(
                    red[:].rearrange("p (k h) -> p k h", k=32),
                    msk4,
                    op=mybir.AluOpType.add,
                    axis=mybir.AxisListType.X,
                )
                # scol[:, b*64 : b*64+64] = red + absel + q (q broadcast on h)
                nc.vector.tensor_tensor(
                    red[:], red[:], absel_ps[:, b * 64 : (b + 1) * 64],
                    op=mybir.AluOpType.add,
                )
                qv = qps[:]
                qb = bass.AP(
                    qv.tensor, qv.offset + b * 32,
                    [qv.ap[0], [1, 32], [0, 2]],
                )
                nc.vector.tensor_tensor(
                    scol[:, b * 64 : (b + 1) * 64], red[:], qb,
                    op=mybir.AluOpType.add,
                )

        if stage >= 4:
            # single 64KB AllReduce: the cc stream's fixed startup barrier
            # outlasts compute, so splitting buys no overlap and the two
            # ARs would just serialize
            cc_in = drpool.tile([128, 2 * NCH], F32, name="cc_in")
            cc_out = drpool.tile([128, 2 * NCH], F32, name="cc_out")
            nc.sync.dma_start(cc_in[:], scol[:])
            if stage >= 5:
                nc.gpsimd.collective_compute(
                    "AllReduce",
                    mybir.AluOpType.add,
                    replica_groups=[list(range(n_cores))],
                    ins=[cc_in.opt()],
                    outs=[cc_out.opt()],
                )
            else:
                nc.sync.dma_start(cc_out[:], cc_in[:])
            # ---------- tail ----------
            sres = spool.tile([128, 2 * NCH], F32, tag="sres")
            nc.sync.dma_start(sres[:], cc_out[:])
            s3 = sres[:].rearrange("p (ci h) -> p ci h", h=2)
            # dbg ships interleaved [p, ci*2+h]; host de-interleaves
            nc.sync.dma_start(dbg_dram.ap(), sres[:])
            rt = spool.tile([128, 2 * NCH], F32, tag="rt")
            nc.scalar.sqrt(rt[:], sres[:])
            rt3 = rt[:].rearrange("p (ci h) -> p ci h", h=2)
            num = spool.tile([128, NCH], F32, tag="num")
            den = spool.tile([128, NCH], F32, tag="den")
            nc.vector.tensor_tensor(
                num[:], s3[:, :, 0:1], s3[:, :, 1:2],
                op=mybir.AluOpType.subtract,
            )
            nc.vector.tensor_tensor(
                den[:], rt3[:, :, 0:1], rt3[:, :, 1:2],
                op=mybir.AluOpType.add,
            )
            rden = spool.tile([128, NCH], F32, tag="rden")
            nc.vector.reciprocal(rden[:], den[:])
            delta = spool.tile([128, NCH], F32, tag="delta")
            nc.vector.tensor_tensor(
                delta[:], num[:], rden[:], op=mybir.AluOpType.mult
            )
            terms = spool.tile([128, NCH], F32, tag="terms")
            lcol = spool.tile([128, 1], F32, tag="lcol")
            margin_t = spool.tile([128, 1], F32, tag="margin")
            nc.vector.memset(margin_t[:], MARGIN)
            nc.scalar.activation(
                terms[:],
                delta[:],
                mybir.ActivationFunctionType.Relu,
                bias=margin_t[:, 0:1],
                scale=1.0,
                accum_out=lcol[:, 0:1],
            )
            loss_ps = ppool2.tile([1, 1], F32, tag="loss")
            nc.tensor.matmul(
                loss_ps[:], lhsT=ones_f[:, 0:1], rhs=lcol[:, 0:1],
                start=True, stop=True,
            )
            nc.scalar.mul(out_sb[:], loss_ps[:], 1.0 / B)
        else:
            nc.scalar.mul(out_sb[:], scol[0:1, 0:1], 1.0)
            nc.sync.dma_start(dbg_dram.ap(), scol[:])

        nc.sync.dma_start(loss_dram.ap(), out_sb[:])

    nc.compile()
    return nc


def build_general(stage=5, n_cores=N_CORES):
    """Correct for arbitrary targets in [0, C)."""
    nc = bacc.Bacc(None, target_bir_lowering=False, debug=False, num_devices=n_cores)
    x_dram = nc.dram_tensor("x", [B, DS], F32, kind="ExternalInput")
    tgt_dram = nc.dram_tensor("tgt", [128, NCH], I32, kind="ExternalInput")
    loss_dram = nc.dram_tensor("loss", [1, 1], F32, kind="ExternalOutput")
    dbg_dram = nc.dram_tensor("dbg", [128, 2 * NCH], F32, kind="ExternalOutput")

    with tile.TileContext(nc) as tc, ExitStack() as top:
        cpool = top.enter_context(tc.tile_pool(name="const", bufs=1))
        ohpool = top.enter_context(tc.tile_pool(name="oh", bufs=4))
        gpool = top.enter_context(tc.tile_pool(name="gath", bufs=4))
        dpool = top.enter_context(tc.tile_pool(name="diff", bufs=3))
        spool = top.enter_context(tc.tile_pool(name="small", bufs=1))
        ppool2 = top.enter_context(tc.tile_pool(name="psum2", bufs=1, space="PSUM"))
        drpool = top.enter_context(tc.tile_pool(name="dram", bufs=1, space="DRAM"))

        xres = cpool.tile([128, NCH * CW], F32, tag="xres")
        x3 = xres[:].rearrange("p (c w) -> p c w", w=CW)
        nc.vector.memset(x3[:, :, DS : DS + 1], 1.0)
        iota_t = cpool.tile([128, C], F32, tag="iota")
        nc.gpsimd.iota(
            iota_t[:], pattern=[[1, C]], base=0, channel_multiplier=0,
            allow_small_or_imprecise_dtypes=True,
        )
        tg32 = cpool.tile([128, NCH], I32, tag="tg32")
        nc.sync.dma_start(tg32[:], tgt_dram.ap())
        tgf = cpool.tile([128, NCH], F32, tag="tgf")
        nc.vector.tensor_copy(tgf[:], tg32[:])
        ones_col = cpool.tile([128, 1], F32, tag="ones_col")
        nc.vector.memset(ones_col[:], 1.0)
        ones_row = cpool.tile([1, 128], F32, tag="ones_row")
        nc.vector.memset(ones_row[:], 1.0)

        xr = x_dram.ap().rearrange("(c p) d -> p c d", p=128)
        for g in range(8):
            nc.sync.dma_start(
                x3[:, g * 8 : (g + 1) * 8, 0:DS], xr[:, g * 8 : (g + 1) * 8, :]
            )

        scol = spool.tile([128, 2 * NCH], F32, tag="scol")
        out_sb = spool.tile([1, 1], F32, tag="out_sb")

        with ExitStack() as ph1:
            ppool1 = ph1.enter_context(
                tc.tile_pool(name="psum1", bufs=1, space="PSUM")
            )
            sums_ps = [
                ppool1.tile([128, CW], F32, tag=f"sums{g}", name=f"sums{g}")
                for g in range(CG)
            ]
            for ci in range(NCH):
                a_t = ohpool.tile([128, C], F32, tag="onehot")
                nc.vector.tensor_scalar(
                    a_t[:],
                    iota_t[:],
                    tgf[:, ci : ci + 1],
                    None,
                    mybir.AluOpType.is_equal,
                )
                rhs = xres[:, ci * CW : (ci + 1) * CW]
                for g in range(CG):
                    nc.tensor.matmul(
                        sums_ps[g][:],
                        lhsT=a_t[:, g * 128 : (g + 1) * 128],
                        rhs=rhs,
                        start=(ci == 0),
                        stop=(ci == NCH - 1),
                    )

            cent = [
                spool.tile([128, DS], F32, tag=f"cent{g}", name=f"cent{g}")
                for g in range(CG)
            ]
            rest = [
                spool.tile([128, DS], F32, tag=f"rest{g}", name=f"rest{g}")
                for g in range(CG)
            ]
            recip = [
                spool.tile([128, 1], F32, tag=f"recip{g}", name=f"recip{g}")
                for g in range(CG)
            ]
            for g in range(CG):
                nc.vector.reciprocal(recip[g][:], sums_ps[g][:, DS : DS + 1])
                nc.vector.tensor_scalar(
                    cent[g][:],
                    sums_ps[g][:, 0:DS],
                    recip[g][:, 0:1],
                    None,
                    mybir.AluOpType.mult,
                )
            tot_ps = ppool1.tile([1, DS], F32, tag="tot")
            for g in range(CG):
                nc.tensor.matmul(
                    tot_ps[:],
                    lhsT=ones_col[:, 0:1],
                    rhs=cent[g][:],
                    start=(g == 0),
                    stop=(g == CG - 1),
                )
            tot_sb = spool.tile([1, DS], F32, tag="tot_sb")
            nc.scalar.mul(tot_sb[:], tot_ps[:], 1.0 / (C - 1))
            tb_ps = ppool1.tile([128, DS], F32, tag="tb")
            nc.tensor.matmul(
                tb_ps[:], lhsT=ones_row[:], rhs=tot_sb[:], start=True, stop=True
            )
            resttmp = spool.tile([128, DS], F32, tag="resttmp")
            for g in range(CG):
                nc.vector.tensor_scalar_mul(resttmp[:], cent[g][:], 1.0 / (C - 1))
                nc.vector.tensor_tensor(
                    rest[g][:], tb_ps[:], resttmp[:], op=mybir.AluOpType.subtract
                )
            table = drpool.tile([C, 2 * DS], F32)
            for g in range(CG):
                nc.sync.dma_start(table[g * 128 : (g + 1) * 128, 0:DS], cent[g][:])
                nc.sync.dma_start(
                    table[g * 128 : (g + 1) * 128, DS : 2 * DS], rest[g][:]
                )

        if stage >= 3:
            for ci in range(NCH):
                cg_t = gpool.tile([128, 2 * DS], F32, tag="gath")
                nc.gpsimd.indirect_dma_start(
                    out=cg_t[:],
                    out_offset=None,
                    in_=table[:],
                    in_offset=IndirectOffsetOnAxis(ap=tg32[:, ci : ci + 1], axis=0),
                )
                xch = xres[:, ci * CW : ci * CW + DS]
                dap = dpool.tile([128, DS], F32, tag="dap")
                dan = dpool.tile([128, DS], F32, tag="dan")
                nc.vector.tensor_tensor(
                    dap[:], xch, cg_t[:, 0:DS], op=mybir.AluOpType.subtract
                )
                nc.vector.tensor_tensor(
                    dan[:], xch, cg_t[:, DS : 2 * DS], op=mybir.AluOpType.subtract
                )
                nc.scalar.activation(
                    dap[:],
                    dap[:],
                    mybir.ActivationFunctionType.Square,
                    accum_out=scol[:, 2 * ci : 2 * ci + 1],
                )
                nc.scalar.activation(
                    dan[:],
                    dan[:],
                    mybir.ActivationFunctionType.Square,
                    accum_out=scol[:, 2 * ci + 1 : 2 * ci + 2],
                )

        if stage >= 4:
            cc_out1 = _ar(nc, drpool, scol[:, 0:NCH], n_cores, stage, "h1")
            cc_out2 = _ar(
                nc, drpool, scol[:, NCH : 2 * NCH], n_cores, stage, "h2"
            )
            _loss_tail(
                nc, spool, ppool2, drpool, dbg_dram, out_sb, cc_out1, cc_out2,
                stage,
            )
        else:
            nc.scalar.mul(out_sb[:], scol[0:1, 0:1], 1.0)
            nc.sync.dma_start(dbg_dram.ap()[:, 0:NCH], scol[:, 0:NCH])
            nc.sync.dma_start(
                dbg_dram.ap()[:, NCH : 2 * NCH], scol[:, NCH : 2 * NCH]
            )

        nc.sync.dma_start(loss_dram.ap(), out_sb[:])

    nc.compile()
    return nc


_PROGRAMS = {}


def _get_program(sorted_fast):
    if sorted_fast not in _PROGRAMS:
        _PROGRAMS[sorted_fast] = (
            build_sorted() if sorted_fast else build_general()
        )
    return _PROGRAMS[sorted_fast]


def is_sorted_balanced(t):
    return bool(np.array_equal(t, np.arange(B, dtype=np.int64) // K))


def make_in_maps(inputs, targets, sorted_fast):
    import ml_dtypes

    x = np.asarray(inputs, dtype=np.float32)
    assert x.shape == (B, D), x.shape
    if sorted_fast:
        # feature-major bf16: xt[p, fb*B + b] = x[b, c*DS + fb*128 + p]
        xm = x.astype(ml_dtypes.bfloat16)
        maps = []
        for c in range(N_CORES):
            xs = xm[:, c * DS : (c + 1) * DS].T  # [DS, B]
            xs = np.ascontiguousarray(
                xs.reshape(2, 128, B).transpose(1, 0, 2).reshape(128, 2 * B)
            )
            maps.append({"xt": xs})
        return maps
    t = np.asarray(targets).astype(np.int32)
    tgt_re = np.ascontiguousarray(t.reshape(NCH, 128).T)  # [128, NCH]
    return [
        {
            "x": np.ascontiguousarray(x[:, c * DS : (c + 1) * DS]),
            "tgt": tgt_re,
        }
        for c in range(N_CORES)
    ]


def kernel(inputs, targets, num_classes, **_unused):
    assert int(num_classes) == C
    sf = is_sorted_balanced(np.asarray(targets))
    nc = _get_program(sf)
    in_maps = make_in_maps(inputs, targets, sf)
    res = run_bass_kernel_spmd(nc, in_maps, core_ids=list(range(N_CORES)))
    val = np.float32(res.results[0]["loss"][0, 0])
    return np.asarray(val, dtype=np.float32).reshape(())

